# revision 14
# baseline (speedup 1.0000x reference)
"""Trainium2 Bass/Tile kernel for nn_MultiHeadAttention (B=2, S=2048, D=1024, H=16).

Sharding (8 NeuronCores): data-parallel over batch (2) x tensor-parallel over
head groups (4 heads per core).  Core c handles batch c//4, heads
[(c%4)*4, (c%4)*4+4).  Each core:

  phase 1: q/k projections in transposed layout qT/kT [256, 2048]
           (j = head-local output channel on partitions, sequence on free),
           v projection in natural layout augmented with a ones column
           (v_aug [s, 65] blocks) so the attn@v matmul also produces the
           softmax denominator row for free.
  phase 2: scores = qT.T-slices @ kT (PE), causal prefix only; exp via
           ScalarE with row-sum accumulation; normalize via VectorE;
           write the normalized attention rows straight to HBM.
  phase 3: scoresT (transposed orientation, so softmax numerators land with
           the key index on partitions), exp, then ctxT[dk, i] accumulation
           on PE with the ones row yielding Z per query column; normalize
           ctxT by 1/Z (outer-product replicate + VectorE multiply).
  phase 4: partial output projection out_part = ctx @ Wo[:, jsel].T (PE).

Host: pre-transposes inputs/weights, folds the 1/sqrt(dk) scale into Wq/bq,
sums the 4 row-parallel out partials per batch, adds bo, and reassembles
attn.  Softmax is computed without max-subtraction (exp(s)/sum exp(s)):
scores for this problem's data are O(10), far inside fp32 exp range, and
softmax is shift-invariant so results match the reference within fp32
rounding.

Mask handling: the mask input is inspected on the host.  Causal (tril) and
all-ones masks use fast specializations (compile-time structure); anything
else falls back to a generic additive-bias path that streams the mask from
HBM.  Masked positions produce exactly 0.0 in attn, matching the reference
(exp(-1e9 - max) underflows to 0).
"""

import os
import math
import numpy as np
from contextlib import ExitStack

import concourse.bass as bass
import concourse.bacc as bacc
import concourse.tile as tile
import concourse.mybir as mybir
from concourse.bass_utils import run_bass_kernel_spmd

F32 = mybir.dt.float32
F32R = mybir.dt.float32r

# Problem constants (hardcoded per contract).
B, S, D, H = 2, 2048, 1024, 16
DK = D // H                 # 64
NCORES = 8
HPC = 4                     # heads per core
JD = HPC * DK               # 256 projected channels per core
NT = S // 128               # 16 row tiles
NEG = -1.0e9

USE_F32R = True             # fp32r matmuls (4x PE throughput vs fp32)
MMDT = F32R if USE_F32R else F32   # dtype for every matmul operand


def _r(ap):
    return ap


def _build_program(mode: str):
    """Build + compile the SPMD Bass program.  mode: 'causal'|'full'|'generic'."""
    causal = mode == "causal"
    generic = mode == "generic"

    nc = bacc.Bacc("TRN2", target_bir_lowering=False, debug=False,
                   enable_asserts=False)

    # ---- DRAM I/O (per core) ----
    qt_d = nc.dram_tensor("QT", [D, S], MMDT, kind="ExternalInput").ap()
    kt_d = nc.dram_tensor("KT", [D, S], MMDT, kind="ExternalInput").ap()
    vt_d = nc.dram_tensor("VT", [D, S], MMDT, kind="ExternalInput").ap()
    wqt_d = nc.dram_tensor("WQT", [D, JD], MMDT, kind="ExternalInput").ap()
    wkt_d = nc.dram_tensor("WKT", [D, JD], MMDT, kind="ExternalInput").ap()
    wvt_d = nc.dram_tensor("WVT", [D, JD], MMDT, kind="ExternalInput").ap()
    wot_d = nc.dram_tensor("WOT", [JD, D], MMDT, kind="ExternalInput").ap()
    bq_d = nc.dram_tensor("BQ", [JD, 1], F32, kind="ExternalInput").ap()
    bk_d = nc.dram_tensor("BK", [JD, 1], F32, kind="ExternalInput").ap()
    bv_d = nc.dram_tensor("BV", [1, JD], MMDT, kind="ExternalInput").ap()
    ones_d = nc.dram_tensor("ONES", [128, 128], MMDT, kind="ExternalInput").ap()
    triu_d = nc.dram_tensor("TRIU", [128, 128], F32, kind="ExternalInput").ap()
    tril_d = nc.dram_tensor("TRIL", [128, 128], F32, kind="ExternalInput").ap()
    if generic:
        maskb_d = nc.dram_tensor("MASKB", [S, S], F32, kind="ExternalInput").ap()
        maskbt_d = nc.dram_tensor("MASKBT", [S, S], F32, kind="ExternalInput").ap()
    attn_d = nc.dram_tensor("ATTN", [HPC, S, S], F32, kind="ExternalOutput").ap()
    outp_d = nc.dram_tensor("OUTP", [S, D], F32, kind="ExternalOutput").ap()

    with tile.TileContext(nc) as tc, ExitStack() as ctx:
        # ---- pools ----
        const_p = ctx.enter_context(tc.tile_pool(name="const", bufs=1))
        xt_p = ctx.enter_context(tc.tile_pool(name="xt", bufs=8))
        w_p = ctx.enter_context(tc.tile_pool(name="w", bufs=1))
        qk_p = ctx.enter_context(tc.tile_pool(name="qk", bufs=1))
        attn_p = ctx.enter_context(tc.tile_pool(name="attn", bufs=3))
        expt_p = ctx.enter_context(tc.tile_pool(name="expt", bufs=2))
        stat_p = ctx.enter_context(tc.tile_pool(name="stat", bufs=8))
        rep_p = ctx.enter_context(tc.tile_pool(name="rep", bufs=2))
        out_p = ctx.enter_context(tc.tile_pool(name="outsb", bufs=2))
        mask_p = ctx.enter_context(tc.tile_pool(name="maskg", bufs=2)) if generic else None
        ps_p = ctx.enter_context(tc.tile_pool(name="ps", bufs=2, space="PSUM"))

        # ---- constants ----
        triu_sb = const_p.tile([128, 128], F32)      # phase2 diag bias (col > row)
        nc.sync.dma_start(triu_sb[:], triu_d[:])
        tril_sb = const_p.tile([128, 128], F32)      # phase3 diag bias (row > col)
        nc.sync.dma_start(tril_sb[:], tril_d[:])
        ones_sb = const_p.tile([1, 128], MMDT)
        nc.sync.dma_start(ones_sb[:], ones_d[0:1, :])
        zero_sb = None
        if causal:
            zero_sb = const_p.tile([128, 2048], F32)
            nc.gpsimd.memset(zero_sb[:], 0.0)
        bq_sb = const_p.tile([128, 2], F32)          # [:, jj] = bias for j-tile jj
        bk_sb = const_p.tile([128, 2], F32)
        for jj in range(2):
            nc.sync.dma_start(bq_sb[:, jj:jj + 1], bq_d[128 * jj:128 * jj + 128, :])
            nc.sync.dma_start(bk_sb[:, jj:jj + 1], bk_d[128 * jj:128 * jj + 128, :])
        bv_sb = const_p.tile([1, JD], MMDT)
        nc.sync.dma_start(bv_sb[:], bv_d[:])

        # ---- persistent activations ----
        # qT/kT: [j, s] layout; j-tile jj holds channels [128jj, 128jj+128).
        qt_sb = [qk_p.tile([128, S], MMDT, tag=f"qt{i}", name=f"qt{i}") for i in range(2)]
        kt_sb = [qk_p.tile([128, S], MMDT, tag=f"kt{i}", name=f"kt{i}") for i in range(2)]
        # v_aug: per (jt, h) block of 65 cols: [v(s,dk) | ones]; s-tile jt on
        # partitions.  Block b = 4*jt + h at cols [65b, 65b+65).
        vaug_sb = qk_p.tile([128, 65 * 4 * NT], MMDT, tag="vaug")
        nc.sync.dma_start(
            vaug_sb.rearrange("p (b c) -> p b c", c=65)[:, :, 64:65],
            ones_d[:, 0:64].rearrange("p (b c) -> p b c", c=1))
        # ctxT: [j, s] layout, unnormalized until phase 3 tail.
        ctxt_sb = [qk_p.tile([128, S], MMDT, tag=f"ctxt{i}", name=f"ctxt{i}") for i in range(2)]
        # weights
        wq_sb = [w_p.tile([128, JD], MMDT, tag=f"wq{i}", name=f"wq{i}") for i in range(8)]
        wk_sb = [w_p.tile([128, JD], MMDT, tag=f"wk{i}", name=f"wk{i}") for i in range(8)]
        wv_sb = [w_p.tile([128, JD], MMDT, tag=f"wv{i}", name=f"wv{i}") for i in range(8)]
        wo_sb = [w_p.tile([128, D], MMDT, tag=f"wo{i}", name=f"wo{i}") for i in range(2)]
        for d8 in range(8):
            nc.sync.dma_start(wq_sb[d8][:], wqt_d[128 * d8:128 * d8 + 128, :])
            nc.sync.dma_start(wk_sb[d8][:], wkt_d[128 * d8:128 * d8 + 128, :])
            nc.sync.dma_start(wv_sb[d8][:], wvt_d[128 * d8:128 * d8 + 128, :])
        for jc in range(2):
            nc.sync.dma_start(wo_sb[jc][:], wot_d[128 * jc:128 * jc + 128, :])

        # ================= phase 1: projections =================
        for sc in range(4):                          # 512-col s-chunks
            scol = 512 * sc
            for which, src_d, w_tiles, dst, b_sb in (
                ("q", qt_d, wq_sb, qt_sb, bq_sb),
                ("k", kt_d, wk_sb, kt_sb, bk_sb),
            ):
                ps = ps_p.tile([128, 1024], F32, tag="sc")
                xts = []
                for d8 in range(8):
                    xt = xt_p.tile([128, 512], MMDT, tag="xt")
                    nc.sync.dma_start(xt[:], src_d[128 * d8:128 * d8 + 128,
                                                   scol:scol + 512])
                    xts.append(xt)
                    for jj in range(2):
                        nc.tensor.matmul(
                            ps[:, 512 * jj:512 * jj + 512],
                            _r(w_tiles[d8][:, 128 * jj:128 * jj + 128]),
                            _r(xt[:]),
                            start=(d8 == 0), stop=(d8 == 7))
                for jj in range(2):
                    nc.scalar.activation(
                        dst[jj][:, scol:scol + 512],
                        ps[:, 512 * jj:512 * jj + 512],
                        mybir.ActivationFunctionType.Identity,
                        bias=b_sb[:, jj:jj + 1])
            # v: natural layout [s, j], four 128-row subtiles per chunk
            vts = []
            for d8 in range(8):
                xt = xt_p.tile([128, 512], MMDT, tag="xt")
                nc.sync.dma_start(xt[:], vt_d[128 * d8:128 * d8 + 128,
                                              scol:scol + 512])
                vts.append(xt)
            for ss in range(4):
                jt = 4 * sc + ss
                ps = ps_p.tile([128, 1024], F32, tag="sc")
                for d8 in range(8):
                    nc.tensor.matmul(
                        ps[:, 0:JD],
                        _r(vts[d8][:, 128 * ss:128 * ss + 128]),
                        _r(wv_sb[d8][:]),
                        start=(d8 == 0), stop=False)
                nc.tensor.matmul(ps[:, 0:JD], _r(ones_sb[:, 0:128]),
                                 _r(bv_sb[:]), start=False, stop=True)
                # scatter 4 head blocks into v_aug (65-strided)
                nc.scalar.copy(
                    vaug_sb[:, 65 * 4 * jt:65 * 4 * jt + 260].rearrange(
                        "p (h c) -> p h c", h=4, c=65)[:, :, 0:64],
                    ps[:, 0:JD].rearrange("p (h c) -> p h c", h=4, c=64))

        def q_slice(h, c0, c1):
            jj, po = divmod(h * DK, 128)
            return qt_sb[jj][po:po + DK, c0:c1]

        def k_slice(h, c0, c1):
            jj, po = divmod(h * DK, 128)
            return kt_sb[jj][po:po + DK, c0:c1]

        # ========== phases 2+3, emitted interleaved so PE/ACT/DVE stay dense ==========
        # Phase 2 processes the two heads of one partition tile (jj) as a
        # pair: their lhsT slices sit at base partitions 0 and 64, so the PE
        # runs the two matmuls concurrently in disjoint row groups.
        mrow_cache = {}

        def emit_p2(it, jj):
            hA = 2 * jj
            P = 128 * (it + 1) if causal else S
            ncb = (P + 1023) // 1024
            if generic and jj == 0:
                tiles = []
                for cb in range(2):
                    mt = mask_p.tile([128, 1024], F32, tag="mrow",
                                     name=f"mrow{it}_{cb}")
                    nc.sync.dma_start(
                        mt[:], maskb_d[128 * it:128 * it + 128,
                                       1024 * cb:1024 * cb + 1024])
                    tiles.append(mt)
                mrow_cache[it] = tiles
            ats = [attn_p.tile([128, 2048], F32, tag="attn",
                               name=f"at{it}_{jj}_{hh}") for hh in range(2)]
            zp = stat_p.tile([128, 2], F32, tag="z", name=f"zp{it}_{jj}")
            for cb in range(ncb):
                base = 1024 * cb
                fd = min(1024, P - base)
                pss = [ps_p.tile([128, 1024], F32, tag="sc",
                                 name=f"ps{it}_{jj}_{cb}_{hh}") for hh in range(2)]
                for a in range(0, fd, 512):
                    b_ = min(a + 512, fd)
                    for hh in range(2):
                        nc.tensor.matmul(
                            pss[hh][:, a:b_],
                            q_slice(hA + hh, 128 * it, 128 * it + 128),
                            k_slice(hA + hh, base + a, base + b_),
                            start=True, stop=True)
                if causal and base <= P - 128 < base + fd:
                    dcol = P - 128 - base
                    for hh in range(2):
                        nc.vector.tensor_tensor(
                            pss[hh][:, dcol:dcol + 128],
                            pss[hh][:, dcol:dcol + 128],
                            triu_sb[:], mybir.AluOpType.add)
                if generic:
                    for hh in range(2):
                        nc.vector.tensor_tensor(
                            pss[hh][:, 0:fd], pss[hh][:, 0:fd],
                            mrow_cache[it][cb][:, 0:fd], mybir.AluOpType.add)
                for hh in range(2):
                    if cb == 0:
                        nc.scalar.activation(
                            ats[hh][:, base:base + fd], pss[hh][:, 0:fd],
                            mybir.ActivationFunctionType.Exp,
                            accum_out=zp[:, hh:hh + 1])
                    else:
                        zt = stat_p.tile([128, 1], F32, tag="zt",
                                         name=f"zt{it}_{jj}_{cb}_{hh}")
                        nc.scalar.activation(
                            ats[hh][:, base:base + fd], pss[hh][:, 0:fd],
                            mybir.ActivationFunctionType.Exp, accum_out=zt[:])
                        nc.vector.tensor_tensor(
                            zp[:, hh:hh + 1], zp[:, hh:hh + 1], zt[:],
                            mybir.AluOpType.add)
            rzp = stat_p.tile([128, 2], F32, tag="rz", name=f"rz{it}_{jj}")
            nc.vector.reciprocal(rzp[:], zp[:])
            for hh in range(2):
                at = ats[hh]
                nc.vector.tensor_scalar_mul(at[:, 0:P], at[:, 0:P],
                                            rzp[:, hh:hh + 1])
                nc.sync.dma_start(
                    attn_d[hA + hh][128 * it:128 * it + 128, 0:P], at[:, 0:P])
                if P < S:
                    nc.sync.dma_start(
                        attn_d[hA + hh][128 * it:128 * it + 128, P:S],
                        zero_sb[:, 0:S - P])

        ctxz_state = {}

        def emit_p3_jt(h, c, jt, jts, lastw):
            ci0 = 1024 * c
            if jt == jts[0]:
                ctxz_state[(h, c)] = ps_p.tile([65, 1024], F32, tag="acc",
                                               bufs=1, name=f"ctxz{h}_{c}")
            ctxz = ctxz_state[(h, c)]
            s0 = max(0, 128 * jt - ci0) if causal else 0
            ps2 = ps_p.tile([128, 1024], F32, tag="sct", bufs=1, name=f"sct{h}_{c}_{jt}")
            rngs = [(s0, 512), (512, 1024)] if s0 < 512 else [(s0, 1024)]
            for (a, b_) in rngs:
                nc.tensor.matmul(
                    ps2[:, a:b_],
                    k_slice(h, 128 * jt, 128 * jt + 128),
                    q_slice(h, ci0 + a, ci0 + b_),
                    start=True, stop=True)
            if causal and 128 * jt >= ci0:
                nc.vector.tensor_tensor(
                    ps2[:, s0:s0 + 128], ps2[:, s0:s0 + 128],
                    tril_sb[:], mybir.AluOpType.add)
            if generic:
                mt = mask_p.tile([128, 1024], F32, tag="mtrow",
                                 name=f"mtrow{h}_{c}_{jt}")
                nc.sync.dma_start(
                    mt[:], maskbt_d[128 * jt:128 * jt + 128, ci0:ci0 + 1024])
                nc.vector.tensor_tensor(ps2[:], ps2[:], mt[:],
                                        mybir.AluOpType.add)
            et = expt_p.tile([128, 1024], MMDT, tag="expt",
                             name=f"et{h}_{c}_{jt}")
            nc.scalar.activation(et[:, s0:1024], ps2[:, s0:1024],
                                 mybir.ActivationFunctionType.Exp)
            vb = 65 * (4 * jt + h)
            for (a, b_) in rngs:
                bank = 0 if a < 512 else 1
                nc.tensor.matmul(
                    ctxz[:, a:b_], vaug_sb[:, vb:vb + 65], et[:, a:b_],
                    start=(jt == 0), stop=(lastw[bank] == jt),
                    skip_group_check=True)

        def emit_p3_tail(h, c):
            ctxz = ctxz_state.pop((h, c))
            ci0 = 1024 * c
            rzr = stat_p.tile([1, 1024], MMDT, tag="rzr", bufs=2, name=f"rzr{h}_{c}")
            with nc.allow_low_precision(reason="1/Z feeds an fp32r matmul "
                                        "which rounds operands anyway"):
                nc.vector.reciprocal(rzr[:], ctxz[64:65, :])
            repps = ps_p.tile([128, 1024], F32, tag="sct", bufs=1, name=f"rep{h}_{c}")
            for a in (0, 512):
                nc.tensor.matmul(repps[0:64, a:a + 512], ones_sb[:, 0:64],
                                 rzr[:, a:a + 512], start=True, stop=True)
            rep_sb = rep_p.tile([64, 1024], F32, tag="rep", name=f"repsb{h}_{c}")
            nc.scalar.copy(rep_sb[:], repps[0:64, :])
            jj, po = divmod(h * DK, 128)
            nc.vector.tensor_tensor(
                ctxt_sb[jj][po:po + DK, ci0:ci0 + 1024],
                ctxz[0:64, :], rep_sb[:], mybir.AluOpType.mult)

        p2_units = [(it, jj) for it in range(NT) for jj in range(2)]
        p3_units = []
        for h in range(HPC):
            for c in range(2):
                jts = list(range(8 * c + 8)) if causal else list(range(NT))
                lastw = {}
                for jt in jts:
                    s0 = max(0, 128 * jt - 1024 * c) if causal else 0
                    if s0 < 512:
                        lastw[0] = jt
                    lastw[1] = jt
                for jt in jts:
                    p3_units.append(("jt", h, c, jt, jts, lastw))
                p3_units.append(("tail", h, c))
        n2, n3 = len(p2_units), len(p3_units)
        i2 = 0
        for i3, u in enumerate(p3_units):
            if u[0] == "jt":
                emit_p3_jt(u[1], u[2], u[3], u[4], u[5])
            else:
                emit_p3_tail(u[1], u[2])
            while i2 * n3 < (i3 + 1) * n2 and i2 < n2:
                emit_p2(*p2_units[i2])
                i2 += 1
        while i2 < n2:
            emit_p2(*p2_units[i2])
            i2 += 1

        # ================= phase 4: output projection (partial) =================
        for it in range(NT):
            pst = ps_p.tile([128, 1024], F32, tag="sc")
            obt = out_p.tile([128, 1024], F32, tag="outsb")
            for bank in range(2):
                a = 512 * bank
                for jc in range(2):
                    nc.tensor.matmul(
                        pst[:, a:a + 512],
                        ctxt_sb[jc][:, 128 * it:128 * it + 128],
                        wo_sb[jc][:, a:a + 512],
                        start=(jc == 0), stop=(jc == 1))
            nc.vector.tensor_copy(obt[:], pst[:])
            nc.sync.dma_start(outp_d[128 * it:128 * it + 128, :], obt[:])

    nc.compile()
    return nc


_PROGRAMS: dict = {}


def _get_program(mode: str):
    if mode not in _PROGRAMS:
        _PROGRAMS[mode] = _build_program(mode)
    return _PROGRAMS[mode]


def _mask_mode(mask2d: np.ndarray) -> str:
    if mask2d.all():
        return "full"
    if np.array_equal(mask2d, np.tril(np.ones((S, S), dtype=bool))):
        return "causal"
    return "generic"


def _tri_bias():
    r = np.arange(128)
    triu = np.where(r[None, :] > r[:, None], np.float32(NEG), np.float32(0.0))
    tril = np.where(r[:, None] > r[None, :], np.float32(NEG), np.float32(0.0))
    return np.ascontiguousarray(triu, np.float32), np.ascontiguousarray(tril, np.float32)


def kernel(Q, K, V, mask, Wq, bq, Wk, bk, Wv, bv, Wo, bo):
    Q, K, V = (np.asarray(x, np.float32) for x in (Q, K, V))
    Wq, Wk, Wv, Wo = (np.asarray(x, np.float32) for x in (Wq, Wk, Wv, Wo))
    bq, bk, bv, bo = (np.asarray(x, np.float32) for x in (bq, bk, bv, bo))
    mask2d = np.asarray(mask).reshape(S, S).astype(bool)

    mode = _mask_mode(mask2d)
    nc = _get_program(mode)

    scale = np.float32(1.0 / math.sqrt(DK))
    triu_b, tril_b = _tri_bias()
    if mode == "generic":
        maskb = np.where(mask2d, np.float32(0.0), np.float32(NEG))
        maskbt = np.ascontiguousarray(maskb.T)

    in_maps = []
    for core in range(NCORES):
        b = core // HPC
        hg = core % HPC
        jsel = slice(hg * JD, hg * JD + JD)
        m = {
            "QT": np.ascontiguousarray(Q[b].T),
            "KT": np.ascontiguousarray(K[b].T),
            "VT": np.ascontiguousarray(V[b].T),
            "WQT": np.ascontiguousarray((Wq[jsel] * scale).T),
            "WKT": np.ascontiguousarray(Wk[jsel].T),
            "WVT": np.ascontiguousarray(Wv[jsel].T),
            "WOT": np.ascontiguousarray(Wo[:, jsel].T),
            "BQ": np.ascontiguousarray((bq[jsel] * scale).reshape(JD, 1)),
            "BK": np.ascontiguousarray(bk[jsel].reshape(JD, 1)),
            "BV": np.ascontiguousarray(bv[jsel].reshape(1, JD)),
            "ONES": np.ones((128, 128), np.float32),
            "TRIU": triu_b,
            "TRIL": tril_b,
        }
        if mode == "generic":
            m["MASKB"] = maskb
            m["MASKBT"] = maskbt
        in_maps.append(m)

    res = run_bass_kernel_spmd(nc, in_maps, core_ids=list(range(NCORES)))

    out = np.zeros((B, S, D), np.float32)
    attn = np.empty((B, H, S, S), np.float32)
    for core in range(NCORES):
        b = core // HPC
        hg = core % HPC
        attn[b, hg * HPC:hg * HPC + HPC] = res.results[core]["ATTN"]
        out[b] += res.results[core]["OUTP"]
    out += bo
    return out, attn


# revision 16
# speedup vs baseline: 1.0798x; 1.0798x over previous
"""Trainium2 Bass/Tile kernel for nn_MultiHeadAttention (B=2, S=2048, D=1024, H=16).

Sharding (8 NeuronCores): data-parallel over batch (2) x tensor-parallel over
head groups (4 heads per core).  Core c handles batch c//4, heads
[(c%4)*4, (c%4)*4+4).  Each core:

  phase 1: q/k projections in transposed layout qT/kT [256, 2048]
           (j = head-local output channel on partitions, sequence on free),
           v projection in natural layout augmented with a ones column
           (v_aug [s, 65] blocks) so the attn@v matmul also produces the
           softmax denominator row for free.
  phase 2: scores = qT.T-slices @ kT (PE), causal prefix only; exp via
           ScalarE with row-sum accumulation; normalize via VectorE;
           write the normalized attention rows straight to HBM.
  phase 3: scoresT (transposed orientation, so softmax numerators land with
           the key index on partitions), exp, then ctxT[dk, i] accumulation
           on PE with the ones row yielding Z per query column; normalize
           ctxT by 1/Z (outer-product replicate + VectorE multiply).
  phase 4: partial output projection out_part = ctx @ Wo[:, jsel].T (PE).

Host: pre-transposes inputs/weights, folds the 1/sqrt(dk) scale into Wq/bq,
sums the 4 row-parallel out partials per batch, adds bo, and reassembles
attn.  Softmax is computed without max-subtraction (exp(s)/sum exp(s)):
scores for this problem's data are O(10), far inside fp32 exp range, and
softmax is shift-invariant so results match the reference within fp32
rounding.

Mask handling: the mask input is inspected on the host.  Causal (tril) and
all-ones masks use fast specializations (compile-time structure); anything
else falls back to a generic additive-bias path that streams the mask from
HBM.  Masked positions produce exactly 0.0 in attn, matching the reference
(exp(-1e9 - max) underflows to 0).
"""

import os
import math
import numpy as np
from contextlib import ExitStack

import concourse.bass as bass
import concourse.bacc as bacc
import concourse.tile as tile
import concourse.mybir as mybir
from concourse.bass_utils import run_bass_kernel_spmd

F32 = mybir.dt.float32
F32R = mybir.dt.float32r

# Problem constants (hardcoded per contract).
B, S, D, H = 2, 2048, 1024, 16
DK = D // H                 # 64
NCORES = 8
HPC = 4                     # heads per core
JD = HPC * DK               # 256 projected channels per core
NT = S // 128               # 16 row tiles
NEG = -1.0e9

USE_F32R = True             # fp32r matmuls (4x PE throughput vs fp32)
MMDT = F32R if USE_F32R else F32   # dtype for every matmul operand


def _r(ap):
    return ap


def _build_program(mode: str):
    """Build + compile the SPMD Bass program.  mode: 'causal'|'full'|'generic'."""
    causal = mode == "causal"
    generic = mode == "generic"

    nc = bacc.Bacc("TRN2", target_bir_lowering=False, debug=False,
                   enable_asserts=False)

    # ---- DRAM I/O (per core) ----
    qt_d = nc.dram_tensor("QT", [D, S], MMDT, kind="ExternalInput").ap()
    kt_d = nc.dram_tensor("KT", [D, S], MMDT, kind="ExternalInput").ap()
    vt_d = nc.dram_tensor("VT", [D, S], MMDT, kind="ExternalInput").ap()
    wqt_d = nc.dram_tensor("WQT", [D, JD], MMDT, kind="ExternalInput").ap()
    wkt_d = nc.dram_tensor("WKT", [D, JD], MMDT, kind="ExternalInput").ap()
    wvt_d = nc.dram_tensor("WVT", [D, JD], MMDT, kind="ExternalInput").ap()
    wot_d = nc.dram_tensor("WOT", [JD, D], MMDT, kind="ExternalInput").ap()
    bq_d = nc.dram_tensor("BQ", [JD, 1], F32, kind="ExternalInput").ap()
    bk_d = nc.dram_tensor("BK", [JD, 1], F32, kind="ExternalInput").ap()
    bv_d = nc.dram_tensor("BV", [1, JD], MMDT, kind="ExternalInput").ap()
    ones_d = nc.dram_tensor("ONES", [128, 128], MMDT, kind="ExternalInput").ap()
    triu_d = nc.dram_tensor("TRIU", [128, 128], F32, kind="ExternalInput").ap()
    tril_d = nc.dram_tensor("TRIL", [128, 128], F32, kind="ExternalInput").ap()
    if generic:
        maskb_d = nc.dram_tensor("MASKB", [S, S], F32, kind="ExternalInput").ap()
        maskbt_d = nc.dram_tensor("MASKBT", [S, S], F32, kind="ExternalInput").ap()
    attn_d = nc.dram_tensor("ATTN", [HPC, S, S], F32, kind="ExternalOutput").ap()
    outp_d = nc.dram_tensor("OUTP", [S, D], F32, kind="ExternalOutput").ap()

    with tile.TileContext(nc) as tc, ExitStack() as ctx:
        # ---- pools ----
        const_p = ctx.enter_context(tc.tile_pool(name="const", bufs=1))
        xt_p = ctx.enter_context(tc.tile_pool(name="xt", bufs=8))
        w_p = ctx.enter_context(tc.tile_pool(name="w", bufs=1))
        qk_p = ctx.enter_context(tc.tile_pool(name="qk", bufs=1))
        attn_p = ctx.enter_context(tc.tile_pool(name="attn", bufs=3))
        expt_p = ctx.enter_context(tc.tile_pool(name="expt", bufs=4))
        stat_p = ctx.enter_context(tc.tile_pool(name="stat", bufs=8))
        rep_p = ctx.enter_context(tc.tile_pool(name="rep", bufs=2))
        out_p = ctx.enter_context(tc.tile_pool(name="outsb", bufs=2))
        mask_p = ctx.enter_context(tc.tile_pool(name="maskg", bufs=2)) if generic else None
        ps_p = ctx.enter_context(tc.tile_pool(name="ps", bufs=2, space="PSUM"))

        # ---- constants ----
        triu_sb = const_p.tile([128, 128], F32)      # phase2 diag bias (col > row)
        nc.sync.dma_start(triu_sb[:], triu_d[:])
        tril_sb = const_p.tile([128, 128], F32)      # phase3 diag bias (row > col)
        nc.sync.dma_start(tril_sb[:], tril_d[:])
        ones_sb = const_p.tile([1, 128], MMDT)
        nc.sync.dma_start(ones_sb[:], ones_d[0:1, :])
        zero_sb = None
        if causal:
            zero_sb = const_p.tile([128, 2048], F32)
            nc.gpsimd.memset(zero_sb[:], 0.0)
        bq_sb = const_p.tile([128, 2], F32)          # [:, jj] = bias for j-tile jj
        bk_sb = const_p.tile([128, 2], F32)
        for jj in range(2):
            nc.sync.dma_start(bq_sb[:, jj:jj + 1], bq_d[128 * jj:128 * jj + 128, :])
            nc.sync.dma_start(bk_sb[:, jj:jj + 1], bk_d[128 * jj:128 * jj + 128, :])
        bv_sb = const_p.tile([1, JD], MMDT)
        nc.sync.dma_start(bv_sb[:], bv_d[:])

        # ---- persistent activations ----
        # qT/kT: [j, s] layout; j-tile jj holds channels [128jj, 128jj+128).
        qt_sb = [qk_p.tile([128, S], MMDT, tag=f"qt{i}", name=f"qt{i}") for i in range(2)]
        kt_sb = [qk_p.tile([128, S], MMDT, tag=f"kt{i}", name=f"kt{i}") for i in range(2)]
        # v_aug: per (jt, h) block of 65 cols: [v(s,dk) | ones]; s-tile jt on
        # partitions.  Block b = 4*jt + h at cols [65b, 65b+65).
        vaug_sb = qk_p.tile([128, 65 * 4 * NT], MMDT, tag="vaug")
        nc.sync.dma_start(
            vaug_sb.rearrange("p (b c) -> p b c", c=65)[:, :, 64:65],
            ones_d[:, 0:64].rearrange("p (b c) -> p b c", c=1))
        # ctxT: [j, s] layout, unnormalized until phase 3 tail.
        ctxt_sb = [qk_p.tile([128, S], MMDT, tag=f"ctxt{i}", name=f"ctxt{i}") for i in range(2)]
        # weights
        wq_sb = [w_p.tile([128, JD], MMDT, tag=f"wq{i}", name=f"wq{i}") for i in range(8)]
        wk_sb = [w_p.tile([128, JD], MMDT, tag=f"wk{i}", name=f"wk{i}") for i in range(8)]
        wv_sb = [w_p.tile([128, JD], MMDT, tag=f"wv{i}", name=f"wv{i}") for i in range(8)]
        wo_sb = [w_p.tile([128, D], MMDT, tag=f"wo{i}", name=f"wo{i}") for i in range(2)]
        for d8 in range(8):
            nc.sync.dma_start(wq_sb[d8][:], wqt_d[128 * d8:128 * d8 + 128, :])
            nc.sync.dma_start(wk_sb[d8][:], wkt_d[128 * d8:128 * d8 + 128, :])
            nc.sync.dma_start(wv_sb[d8][:], wvt_d[128 * d8:128 * d8 + 128, :])
        for jc in range(2):
            nc.sync.dma_start(wo_sb[jc][:], wot_d[128 * jc:128 * jc + 128, :])

        # ================= phase 1: projections =================
        for sc in range(4):                          # 512-col s-chunks
            scol = 512 * sc
            for which, src_d, w_tiles, dst, b_sb in (
                ("q", qt_d, wq_sb, qt_sb, bq_sb),
                ("k", kt_d, wk_sb, kt_sb, bk_sb),
            ):
                pss = [ps_p.tile([128, 512], F32, tag="sc", bufs=4,
                                 name=f"p1{which}_{sc}_{jj}") for jj in range(2)]
                for d8 in range(8):
                    xt = xt_p.tile([128, 512], MMDT, tag="xt",
                                   name=f"xt{which}_{sc}_{d8}")
                    nc.sync.dma_start(xt[:], src_d[128 * d8:128 * d8 + 128,
                                                   scol:scol + 512])
                    for jj in range(2):
                        nc.tensor.matmul(
                            pss[jj][:],
                            w_tiles[d8][:, 128 * jj:128 * jj + 128],
                            xt[:],
                            start=(d8 == 0), stop=(d8 == 7))
                for jj in range(2):
                    nc.scalar.activation(
                        dst[jj][:, scol:scol + 512],
                        pss[jj][:],
                        mybir.ActivationFunctionType.Identity,
                        bias=b_sb[:, jj:jj + 1])
            # v: natural layout [s, j], four 128-row subtiles per chunk
            vts = []
            for d8 in range(8):
                xt = xt_p.tile([128, 512], MMDT, tag="xt",
                               name=f"xtv_{sc}_{d8}")
                nc.sync.dma_start(xt[:], vt_d[128 * d8:128 * d8 + 128,
                                              scol:scol + 512])
                vts.append(xt)
            for ss in range(4):
                jt = 4 * sc + ss
                ps = ps_p.tile([128, 512], F32, tag="sc", bufs=4,
                               name=f"p1v_{sc}_{ss}")
                for d8 in range(8):
                    nc.tensor.matmul(
                        ps[:, 0:JD],
                        vts[d8][:, 128 * ss:128 * ss + 128],
                        wv_sb[d8][:],
                        start=(d8 == 0), stop=False)
                nc.tensor.matmul(ps[:, 0:JD], ones_sb[:, 0:128],
                                 bv_sb[:], start=False, stop=True)
                # scatter 4 head blocks into v_aug (65-strided)
                nc.scalar.copy(
                    vaug_sb[:, 65 * 4 * jt:65 * 4 * jt + 260].rearrange(
                        "p (h c) -> p h c", h=4, c=65)[:, :, 0:64],
                    ps[:, 0:JD].rearrange("p (h c) -> p h c", h=4, c=64))

        def q_slice(h, c0, c1):
            jj, po = divmod(h * DK, 128)
            return qt_sb[jj][po:po + DK, c0:c1]

        def k_slice(h, c0, c1):
            jj, po = divmod(h * DK, 128)
            return kt_sb[jj][po:po + DK, c0:c1]

        # ========== phases 2+3, emitted interleaved so PE/ACT/DVE stay dense ==========
        # Phase 2 processes the two heads of one partition tile (jj) as a
        # pair: their lhsT slices sit at base partitions 0 and 64, so the PE
        # runs the two matmuls concurrently in disjoint row groups.
        mrow_cache = {}

        def emit_p2(it, jj):
            hA = 2 * jj
            P = 128 * (it + 1) if causal else S
            ncb = (P + 511) // 512
            if generic and jj == 0:
                tiles = []
                for cb in range(4):
                    mt = mask_p.tile([128, 512], F32, tag="mrow", bufs=5,
                                     name=f"mrow{it}_{cb}")
                    nc.sync.dma_start(
                        mt[:], maskb_d[128 * it:128 * it + 128,
                                       512 * cb:512 * cb + 512])
                    tiles.append(mt)
                mrow_cache[it] = tiles
            ats = [attn_p.tile([128, 2048], F32, tag="attn",
                               name=f"at{it}_{jj}_{hh}") for hh in range(2)]
            zp = stat_p.tile([128, 2], F32, tag="z", name=f"zp{it}_{jj}")
            for cb in range(ncb):
                base = 512 * cb
                fd = min(512, P - base)
                pss = [ps_p.tile([128, 512], F32, tag="sc", bufs=4,
                                 name=f"ps{it}_{jj}_{cb}_{hh}") for hh in range(2)]
                for hh in range(2):
                    nc.tensor.matmul(
                        pss[hh][:, 0:fd],
                        q_slice(hA + hh, 128 * it, 128 * it + 128),
                        k_slice(hA + hh, base, base + fd),
                        start=True, stop=True)
                if causal and base <= P - 128 < base + fd:
                    dcol = P - 128 - base
                    for hh in range(2):
                        nc.vector.tensor_tensor(
                            pss[hh][:, dcol:dcol + 128],
                            pss[hh][:, dcol:dcol + 128],
                            triu_sb[:], mybir.AluOpType.add)
                if generic:
                    for hh in range(2):
                        nc.vector.tensor_tensor(
                            pss[hh][:, 0:fd], pss[hh][:, 0:fd],
                            mrow_cache[it][cb][:, 0:fd], mybir.AluOpType.add)
                for hh in range(2):
                    if cb == 0:
                        nc.scalar.activation(
                            ats[hh][:, base:base + fd], pss[hh][:, 0:fd],
                            mybir.ActivationFunctionType.Exp,
                            accum_out=zp[:, hh:hh + 1])
                    else:
                        zt = stat_p.tile([128, 1], F32, tag="zt",
                                         name=f"zt{it}_{jj}_{cb}_{hh}")
                        nc.scalar.activation(
                            ats[hh][:, base:base + fd], pss[hh][:, 0:fd],
                            mybir.ActivationFunctionType.Exp, accum_out=zt[:])
                        nc.vector.tensor_tensor(
                            zp[:, hh:hh + 1], zp[:, hh:hh + 1], zt[:],
                            mybir.AluOpType.add)
            rzp = stat_p.tile([128, 2], F32, tag="rz", name=f"rz{it}_{jj}")
            nc.vector.reciprocal(rzp[:], zp[:])
            for hh in range(2):
                at = ats[hh]
                nc.vector.tensor_scalar_mul(at[:, 0:P], at[:, 0:P],
                                            rzp[:, hh:hh + 1])
                nc.sync.dma_start(
                    attn_d[hA + hh][128 * it:128 * it + 128, 0:P], at[:, 0:P])
                if P < S:
                    nc.sync.dma_start(
                        attn_d[hA + hh][128 * it:128 * it + 128, P:S],
                        zero_sb[:, 0:S - P])

        # Phase 3 runs in 512-wide query chunks c (ci0 = 512c); ctxz [65,512]
        # accumulates v_aug.T @ expT with the ones row giving Z per column.
        NC3 = 4
        ctxz_state = {}

        def emit_p3_jt(h, c, jt, jts, lastjt):
            ci0 = 512 * c
            if jt == jts[0]:
                ctxz_state[(h, c)] = ps_p.tile([65, 512], F32, tag="acc",
                                               bufs=2, name=f"ctxz{h}_{c}")
            ctxz = ctxz_state[(h, c)]
            s0 = max(0, 128 * jt - ci0) if causal else 0
            ps2 = ps_p.tile([128, 512], F32, tag="sct", bufs=2,
                            name=f"sct{h}_{c}_{jt}")
            nc.tensor.matmul(
                ps2[:, s0:512],
                k_slice(h, 128 * jt, 128 * jt + 128),
                q_slice(h, ci0 + s0, ci0 + 512),
                start=True, stop=True)
            if causal and 128 * jt >= ci0:
                nc.vector.tensor_tensor(
                    ps2[:, s0:s0 + 128], ps2[:, s0:s0 + 128],
                    tril_sb[:], mybir.AluOpType.add)
            if generic:
                mt = mask_p.tile([128, 512], F32, tag="mtrow",
                                 name=f"mtrow{h}_{c}_{jt}")
                nc.sync.dma_start(
                    mt[:], maskbt_d[128 * jt:128 * jt + 128, ci0:ci0 + 512])
                nc.vector.tensor_tensor(ps2[:], ps2[:], mt[:],
                                        mybir.AluOpType.add)
            et = expt_p.tile([128, 512], MMDT, tag="expt",
                             name=f"et{h}_{c}_{jt}")
            nc.scalar.activation(et[:, s0:512], ps2[:, s0:512],
                                 mybir.ActivationFunctionType.Exp)
            vb = 65 * (4 * jt + h)
            nc.tensor.matmul(
                ctxz[:, s0:512], vaug_sb[:, vb:vb + 65], et[:, s0:512],
                start=(jt == 0), stop=(jt == lastjt),
                skip_group_check=True)

        def emit_p3_tail(h, c):
            ctxz = ctxz_state.pop((h, c))
            ci0 = 512 * c
            # 1/Z row via exp(-ln Z) on ScalarE: streaming LUT ops instead of
            # the DVE's iterative divide (8 cyc/elem on a single partition).
            lnz = stat_p.tile([1, 512], F32, tag="lnz", bufs=2,
                              name=f"lnz{h}_{c}")
            nc.scalar.activation(lnz[:], ctxz[64:65, :],
                                 mybir.ActivationFunctionType.Ln)
            rzr = stat_p.tile([1, 512], MMDT, tag="rzr", bufs=2,
                              name=f"rzr{h}_{c}")
            with nc.allow_low_precision(reason="1/Z feeds an fp32r matmul "
                                        "which rounds operands anyway"):
                nc.scalar.activation(rzr[:], lnz[:],
                                     mybir.ActivationFunctionType.Exp,
                                     scale=-1.0)
            repps = ps_p.tile([128, 512], F32, tag="sct", bufs=2,
                              name=f"rep{h}_{c}")
            nc.tensor.matmul(repps[0:64, :], ones_sb[:, 0:64], rzr[:],
                             start=True, stop=True)
            rep_sb = rep_p.tile([64, 512], F32, tag="rep", name=f"repsb{h}_{c}")
            nc.scalar.copy(rep_sb[:], repps[0:64, :])
            jj, po = divmod(h * DK, 128)
            nc.vector.tensor_tensor(
                ctxt_sb[jj][po:po + DK, ci0:ci0 + 512],
                ctxz[0:64, :], rep_sb[:], mybir.AluOpType.mult)

        p2_units = [(it, jj) for it in range(NT) for jj in range(2)]
        p3_units = []
        for h in range(HPC):
            for c in range(NC3):
                jts = list(range(min(NT, 4 * c + 4))) if causal else list(range(NT))
                for jt in jts:
                    p3_units.append(("jt", h, c, jt, jts, jts[-1]))
                p3_units.append(("tail", h, c))
        n2, n3 = len(p2_units), len(p3_units)
        i2 = 0
        for i3, u in enumerate(p3_units):
            if u[0] == "jt":
                emit_p3_jt(u[1], u[2], u[3], u[4], u[5])
            else:
                emit_p3_tail(u[1], u[2])
            while i2 * n3 < (i3 + 1) * n2 and i2 < n2:
                emit_p2(*p2_units[i2])
                i2 += 1
        while i2 < n2:
            emit_p2(*p2_units[i2])
            i2 += 1

        # ================= phase 4: output projection (partial) =================
        for it in range(NT):
            obt = out_p.tile([128, 1024], F32, tag="outsb")
            for bank in range(2):
                a = 512 * bank
                pst = ps_p.tile([128, 512], F32, tag="sc", bufs=4,
                                name=f"p4_{it}_{bank}")
                for jc in range(2):
                    nc.tensor.matmul(
                        pst[:],
                        ctxt_sb[jc][:, 128 * it:128 * it + 128],
                        wo_sb[jc][:, a:a + 512],
                        start=(jc == 0), stop=(jc == 1))
                nc.vector.tensor_copy(obt[:, a:a + 512], pst[:])
            nc.sync.dma_start(outp_d[128 * it:128 * it + 128, :], obt[:])

    nc.compile()
    return nc


_PROGRAMS: dict = {}


def _get_program(mode: str):
    if mode not in _PROGRAMS:
        _PROGRAMS[mode] = _build_program(mode)
    return _PROGRAMS[mode]


def _mask_mode(mask2d: np.ndarray) -> str:
    if mask2d.all():
        return "full"
    if np.array_equal(mask2d, np.tril(np.ones((S, S), dtype=bool))):
        return "causal"
    return "generic"


def _tri_bias():
    r = np.arange(128)
    triu = np.where(r[None, :] > r[:, None], np.float32(NEG), np.float32(0.0))
    tril = np.where(r[:, None] > r[None, :], np.float32(NEG), np.float32(0.0))
    return np.ascontiguousarray(triu, np.float32), np.ascontiguousarray(tril, np.float32)


def kernel(Q, K, V, mask, Wq, bq, Wk, bk, Wv, bv, Wo, bo):
    Q, K, V = (np.asarray(x, np.float32) for x in (Q, K, V))
    Wq, Wk, Wv, Wo = (np.asarray(x, np.float32) for x in (Wq, Wk, Wv, Wo))
    bq, bk, bv, bo = (np.asarray(x, np.float32) for x in (bq, bk, bv, bo))
    mask2d = np.asarray(mask).reshape(S, S).astype(bool)

    mode = _mask_mode(mask2d)
    nc = _get_program(mode)

    scale = np.float32(1.0 / math.sqrt(DK))
    triu_b, tril_b = _tri_bias()
    if mode == "generic":
        maskb = np.where(mask2d, np.float32(0.0), np.float32(NEG))
        maskbt = np.ascontiguousarray(maskb.T)

    in_maps = []
    for core in range(NCORES):
        b = core // HPC
        hg = core % HPC
        jsel = slice(hg * JD, hg * JD + JD)
        m = {
            "QT": np.ascontiguousarray(Q[b].T),
            "KT": np.ascontiguousarray(K[b].T),
            "VT": np.ascontiguousarray(V[b].T),
            "WQT": np.ascontiguousarray((Wq[jsel] * scale).T),
            "WKT": np.ascontiguousarray(Wk[jsel].T),
            "WVT": np.ascontiguousarray(Wv[jsel].T),
            "WOT": np.ascontiguousarray(Wo[:, jsel].T),
            "BQ": np.ascontiguousarray((bq[jsel] * scale).reshape(JD, 1)),
            "BK": np.ascontiguousarray(bk[jsel].reshape(JD, 1)),
            "BV": np.ascontiguousarray(bv[jsel].reshape(1, JD)),
            "ONES": np.ones((128, 128), np.float32),
            "TRIU": triu_b,
            "TRIL": tril_b,
        }
        if mode == "generic":
            m["MASKB"] = maskb
            m["MASKBT"] = maskbt
        in_maps.append(m)

    res = run_bass_kernel_spmd(nc, in_maps, core_ids=list(range(NCORES)))

    out = np.zeros((B, S, D), np.float32)
    attn = np.empty((B, H, S, S), np.float32)
    for core in range(NCORES):
        b = core // HPC
        hg = core % HPC
        attn[b, hg * HPC:hg * HPC + HPC] = res.results[core]["ATTN"]
        out[b] += res.results[core]["OUTP"]
    out += bo
    return out, attn
